# revision 2
# baseline (speedup 1.0000x reference)
"""BiGAT (2-layer GAT, PyG-style with self-loops) on 8 Trainium2 NeuronCores.

Strategy: partition nodes (and their incoming edges) by destination across 8
cores. Edges are sorted by dst on the host and padded to a uniform
blocks-x-chunks structure so a single SPMD program serves all cores.

Per layer:
  node stage : xh = x @ W (PE), attention dot-products via block-diagonal
               matmul; packed per-node rows [xh | a_src | a_dst | pad] written
               to a local DRAM table; AllGather replicates the table.
  edge stage : dma_gather of full rows by src (gives xh+a_src) and of the
               [a_src|a_dst] suffix by dst-local from the local table;
               e = lrelu(a_src+a_dst); ex = exp(e)  (softmax max-shift is
               skipped -- scores are O(10) so exp cannot overflow, and softmax
               is shift-invariant); msg = xh_src * ex; one-hot matmul
               scatter-adds [msg | ex] into PSUM per 125-node block; epilogue
               divides by the summed ex (denominator), adds bias.

dma_gather constraints honored: int16 indices (src tables split into two
<=25000-row halves; dst uses core-local indices), row strides and elem sizes
multiples of 256B, indices wrapped [16, n/16] and replicated to 128 partitions.
"""
import sys

sys.path.insert(0, "/opt/trn_rl_repo")

import numpy as np

from concourse import bass, mybir
import concourse.bacc as bacc
import concourse.tile as tile
from concourse.masks import make_identity

F32 = mybir.dt.float32
I16 = mybir.dt.int16
BF16 = True                      # table/gather dtype (False -> float32)
TD = mybir.dt.bfloat16 if BF16 else F32
import ml_dtypes
TNP = ml_dtypes.bfloat16 if BF16 else np.float32

# ---------------- problem constants (hardcoded per contract) ----------------
N_NODES = 50000
N_EDGES = 800000
IN_C, HID_C, OUT_C, HEADS = 128, 16, 64, 8
NEG_SLOPE = 0.2
N_CORES = 8

# ---------------- sharding / tiling parameters ----------------
BLK = 125       # dst nodes per edge-stage block (<=128 for one-hot)
P1W = 256 if BF16 else 192   # [xh(128) | a_src(8) | a_dst(8) | pad]
P2W = 128                    # [xh2(64) | a_src2(1) | a_dst2(1) | pad]
G_MAIN = 2      # blocks per gather call (must divide NB)
EPS = 1e-16


def _wrap16(idx):
    """[L] int array -> dma_gather wrapped layout [128, L//16] int16."""
    L = len(idx)
    w = idx.reshape(L // 16, 16).T
    return np.tile(w, (8, 1)).astype(np.int16)


def _host_prep(x, edge_index, W1, att_src1, att_dst1, b1, W2, att_src2, att_dst2, b2,
               n_nodes=N_NODES, n_cores=N_CORES):
    """Sort/pad edges, build per-core input maps and compile-time params."""
    NP = n_nodes // n_cores
    NB = NP // BLK
    assert NB * BLK == NP
    HALF = n_nodes // 2
    assert HALF < 32768 and NP < 32768

    src = np.concatenate([np.asarray(edge_index[0]), np.arange(n_nodes)])
    dst = np.concatenate([np.asarray(edge_index[1]), np.arange(n_nodes)])
    order = np.argsort(dst, kind="stable")
    src = src[order].astype(np.int64)
    dst = dst[order].astype(np.int64)

    nblk_tot = n_cores * NB
    blk_of = dst // BLK
    # within each dst-block, put src<HALF ("a") edges first
    order2 = np.lexsort((src >= HALF, blk_of))
    src, dst = src[order2], dst[order2]
    is_b = src >= HALF
    cnt_a = np.bincount(blk_of[order2], weights=~is_b, minlength=nblk_tot).astype(np.int64)
    cnt_b = np.bincount(blk_of[order2], weights=is_b, minlength=nblk_tot).astype(np.int64)
    starts = np.concatenate([[0], np.cumsum(cnt_a + cnt_b)]).astype(np.int64)
    Ka = int(np.ceil(cnt_a.max() / 128))
    Kb = int(np.ceil(cnt_b.max() / 128))
    K = Ka + Kb

    # per-block padded arrays in [a-pad | b-pad] chunk order
    srcA = np.zeros((nblk_tot, Ka * 128), np.int64)      # pad -> row 0
    srcB = np.zeros((nblk_tot, Kb * 128), np.int64)
    dstL = np.zeros((nblk_tot, K * 128), np.int64)       # dst local to core
    dloc = np.full((nblk_tot, K * 128), 999.0, np.float32)  # dst local to block
    for b in range(nblk_tot):
        na, nb_ = int(cnt_a[b]), int(cnt_b[b])
        s = starts[b]
        core = b // NB
        srcA[b, :na] = src[s:s + na]
        srcB[b, :nb_] = src[s + na:s + na + nb_] - HALF
        dstL[b, :na] = dst[s:s + na] - core * NP
        dstL[b, Ka * 128:Ka * 128 + nb_] = dst[s + na:s + na + nb_] - core * NP
        dloc[b, :na] = dst[s:s + na] - b * BLK
        dloc[b, Ka * 128:Ka * 128 + nb_] = dst[s + na:s + na + nb_] - b * BLK

    # shared (replicated) weights
    AA1 = np.zeros((128, 16), np.float32)
    asrc1 = np.asarray(att_src1, np.float32)
    adst1 = np.asarray(att_dst1, np.float32)
    for h in range(HEADS):
        AA1[16 * h:16 * (h + 1), h] = asrc1[h]
        AA1[16 * h:16 * (h + 1), 8 + h] = adst1[h]
    AA2 = np.stack([np.asarray(att_src2, np.float32)[0],
                    np.asarray(att_dst2, np.float32)[0]], axis=1)  # [64, 2]
    shared = {
        "W1": np.asarray(W1, np.float32),
        "AA1": AA1,
        "B1": np.tile(np.asarray(b1, np.float32), (128, 1)),
        "W2": np.asarray(W2, np.float32),
        "AA2": AA2,
        "B2": np.tile(np.asarray(b2, np.float32), (128, 1)),
        "IOTA": np.tile(np.arange(128), (128, 1)).astype(TNP),
    }

    xT = np.ascontiguousarray(np.asarray(x, np.float32).T)  # [128, N]

    in_maps = []
    for c in range(n_cores):
        lo = c * NB
        # one merged per-block idx tensor: [a-idxs | b-idxs | dst-idxs] wrapped
        idx = np.stack([
            np.concatenate([_wrap16(srcA[lo + b]), _wrap16(srcB[lo + b]),
                            _wrap16(dstL[lo + b])], axis=1)
            for b in range(NB)])
        dl = dloc[lo:lo + NB].reshape(NB, K, 128).transpose(0, 2, 1)
        m = dict(shared)
        m["xT"] = np.ascontiguousarray(xT[:, c * NP:(c + 1) * NP])
        m["IDX"] = np.ascontiguousarray(idx)
        m["DLOC"] = np.ascontiguousarray(dl.astype(TNP))
        in_maps.append(m)

    prm = dict(NP=NP, NB=NB, K=K, Ka=Ka, Kb=Kb,
               n_nodes=n_nodes, n_cores=n_cores, HALF=HALF)
    return in_maps, prm


def _build_program(prm, repeat=1):
    NP, NB, K, Ka, Kb = prm["NP"], prm["NB"], prm["K"], prm["Ka"], prm["Kb"]
    HALF = prm["HALF"]
    n_nodes, n_cores = prm["n_nodes"], prm["n_cores"]
    RG = [list(range(n_cores))]
    CW = (Ka + Kb + K) * 8  # idx tensor cols (wrapped, 8 per chunk)
    CC = 8                  # max chunks (1024 idxs) per dma_gather call

    nc = bacc.Bacc("TRN2", target_bir_lowering=False, debug=False,
                   num_devices=n_cores, num_swdge_queues=4)
    qn = [0]  # round-robin SWDGE queue assignment for gathers

    def next_q():
        qn[0] += 1
        return qn[0] % 4

    # inputs
    xT = nc.dram_tensor("xT", [128, NP], F32, kind="ExternalInput")
    W1 = nc.dram_tensor("W1", [128, 128], F32, kind="ExternalInput")
    AA1 = nc.dram_tensor("AA1", [128, 16], F32, kind="ExternalInput")
    B1 = nc.dram_tensor("B1", [128, 128], F32, kind="ExternalInput")
    W2 = nc.dram_tensor("W2", [128, 64], F32, kind="ExternalInput")
    AA2 = nc.dram_tensor("AA2", [64, 2], F32, kind="ExternalInput")
    B2 = nc.dram_tensor("B2", [128, 64], F32, kind="ExternalInput")
    IOTA = nc.dram_tensor("IOTA", [128, 128], TD, kind="ExternalInput")
    IDX = nc.dram_tensor("IDX", [NB, 128, CW], I16, kind="ExternalInput")
    DLOC = nc.dram_tensor("DLOC", [NB, 128, K], TD, kind="ExternalInput")
    OUT = nc.dram_tensor("out", [NP, OUT_C], F32, kind="ExternalOutput")
    # internal DRAM
    P1L = nc.dram_tensor("P1L", [NP, P1W], TD)
    P1F = nc.dram_tensor("P1F", [n_nodes, P1W], TD, addr_space="Shared")
    P2L = nc.dram_tensor("P2L", [NP, P2W], TD)
    P2F = nc.dram_tensor("P2F", [n_nodes, P2W], TD, addr_space="Shared")

    mm = mybir.AluOpType
    ACT = mybir.ActivationFunctionType

    from contextlib import ExitStack
    with tile.TileContext(nc) as tc, ExitStack() as ctx:
        cst = ctx.enter_context(tc.tile_pool(name="cst", bufs=1))
        W1t = cst.tile([128, 128], F32)
        AA1t = cst.tile([128, 16], F32)
        B1t = cst.tile([128, 128], F32)
        W2t = cst.tile([128, 64], F32)
        AA2t = cst.tile([64, 2], F32)
        B2t = cst.tile([128, 64], F32)
        IOTAt = cst.tile([128, 128], TD)
        IDENT = cst.tile([128, 128], F32)
        for t, d in ((W1t, W1), (AA1t, AA1), (B1t, B1), (W2t, W2),
                     (AA2t, AA2), (B2t, B2), (IOTAt, IOTA)):
            nc.sync.dma_start(out=t[:], in_=d[:, :])
        make_identity(nc, IDENT[:])

        # body may be repeated for differential benchmarking
        for _rep in range(repeat):
            hT, free_hT = tc.tile([128, NP], F32, name="hT")  # h^T, persists L1

            # ---------------- stage A: L1 node stage ----------------
            with tc.tile_pool(name="pa", bufs=3) as pa, \
                 tc.tile_pool(name="ppa", bufs=3, space="PSUM") as ppa:
                for c0 in range(0, NP, 128):
                    nn = min(128, NP - c0)
                    xt = pa.tile([128, 128], F32, tag="xt")
                    nc.sync.dma_start(out=xt[:, :nn], in_=xT[:, c0:c0 + nn])
                    pm = ppa.tile([128, 128], F32, tag="pp")
                    nc.tensor.matmul(pm[:, :nn], lhsT=W1t[:], rhs=xt[:, :nn],
                                     start=True, stop=True)
                    xhT = pa.tile([128, 128], F32, tag="xhT")
                    nc.vector.tensor_copy(out=xhT[:, :nn], in_=pm[:, :nn])
                    pm2 = ppa.tile([16, 128], F32, tag="pp")
                    nc.tensor.matmul(pm2[:, :nn], lhsT=AA1t[:], rhs=xhT[:, :nn],
                                     start=True, stop=True)
                    aaT = pa.tile([16, 128], F32, tag="aaT")
                    nc.vector.tensor_copy(out=aaT[:, :nn], in_=pm2[:, :nn])
                    pt = ppa.tile([128, 128], F32, tag="pp")
                    nc.tensor.transpose(pt[:nn, :], xhT[:, :nn], IDENT[:])
                    xh = pa.tile([128, 128], TD, tag="xh")
                    nc.vector.tensor_copy(out=xh[:nn, :], in_=pt[:nn, :])
                    pt2 = ppa.tile([128, 16], F32, tag="pp")
                    nc.tensor.transpose(pt2[:nn, :], aaT[:, :nn], IDENT[:16, :16])
                    aa = pa.tile([128, P1W - 128], TD, tag="aa")
                    nc.vector.memset(aa[:, 16:], 0.0)
                    nc.vector.tensor_copy(out=aa[:nn, :16], in_=pt2[:nn, :])
                    nc.sync.dma_start(out=P1L[c0:c0 + nn, 0:128], in_=xh[:nn, :])
                    nc.sync.dma_start(out=P1L[c0:c0 + nn, 128:P1W], in_=aa[:nn, :])

            nc.gpsimd.collective_compute(
                "AllGather", mm.bypass, replica_groups=RG,
                ins=[P1L[:, :]], outs=[P1F[:, :]])

            # ---------------- L1 edge stage (+ fused L2 node stage) ----------------
            with tc.tile_pool(name="gma", bufs=2) as gmap, \
                 tc.tile_pool(name="gmb", bufs=2) as gmbp, \
                 tc.tile_pool(name="gad", bufs=2) as gadp, \
                 tc.tile_pool(name="off", bufs=2) as offp, \
                 tc.tile_pool(name="sml", bufs=3) as sml, \
                 tc.tile_pool(name="sp", bufs=3) as spp, \
                 tc.tile_pool(name="hb", bufs=2) as hbp, \
                 tc.tile_pool(name="a2", bufs=2) as a2p, \
                 tc.tile_pool(name="ps1", bufs=2, space="PSUM") as ps1p, \
                 tc.tile_pool(name="psa2", bufs=2, space="PSUM") as psa2p:

                for b in range(NB):
                        ix = offp.tile([128, CW], I16, tag="ix")
                        nc.sync.dma_start(out=ix[:], in_=IDX[b, :, :])
                        gmA = gmap.tile([128, Ka * P1W], TD, tag="gmA")
                        gmA3 = gmA[:].rearrange("p (r w) -> p r w", w=P1W)
                        for c0 in range(0, Ka, CC):
                            c1 = min(c0 + CC, Ka)
                            nc.gpsimd.dma_gather(
                                out_ap=gmA3[:, c0:c1, :], in_ap=P1F[0:HALF, :],
                                idxs_ap=ix[:, c0 * 8:c1 * 8],
                                num_idxs=(c1 - c0) * 128,
                                num_idxs_reg=(c1 - c0) * 128, elem_size=P1W, queue_num=next_q())
                        gmB = gmbp.tile([128, Kb * P1W], TD, tag="gmB")
                        gmB3 = gmB[:].rearrange("p (r w) -> p r w", w=P1W)
                        for c0 in range(0, Kb, CC):
                            c1 = min(c0 + CC, Kb)
                            nc.gpsimd.dma_gather(
                                out_ap=gmB3[:, c0:c1, :],
                                in_ap=P1F[HALF:n_nodes, :],
                                idxs_ap=ix[:, (Ka + c0) * 8:(Ka + c1) * 8],
                                num_idxs=(c1 - c0) * 128,
                                num_idxs_reg=(c1 - c0) * 128, elem_size=P1W, queue_num=next_q())
                        gad = gadp.tile([128, K * (P1W - 128)], TD, tag="gad")
                        gad3 = gad[:].rearrange("p (r w) -> p r w", w=P1W - 128)
                        for c0 in range(0, K, CC):
                            c1 = min(c0 + CC, K)
                            nc.gpsimd.dma_gather(
                                out_ap=gad3[:, c0:c1, :], in_ap=P1L[:, 128:P1W],
                                idxs_ap=ix[:, (Ka + Kb + c0) * 8:(Ka + Kb + c1) * 8],
                                num_idxs=(c1 - c0) * 128,
                                num_idxs_reg=(c1 - c0) * 128, elem_size=P1W - 128,
                                elem_step=P1W, queue_num=next_q())
                        bi = 0
                        dl = sml.tile([128, K], TD, tag="dl")
                        nc.sync.dma_start(out=dl[:], in_=DLOC[b, :, :])
                        ea = sml.tile([128, K * 8], F32, tag="ea")
                        ea3 = ea[:].rearrange("p (r w) -> p r w", w=8)
                        # e = a_src[src] + a_dst[dst]; a-chunks then b-chunks
                        nc.vector.tensor_tensor(
                            out=ea3[:, 0:Ka, :],
                            in0=gmA3[:, bi * Ka:(bi + 1) * Ka, 128:136],
                            in1=gad3[:, bi * K:bi * K + Ka, 8:16], op=mm.add)
                        nc.vector.tensor_tensor(
                            out=ea3[:, Ka:K, :],
                            in0=gmB3[:, bi * Kb:(bi + 1) * Kb, 128:136],
                            in1=gad3[:, bi * K + Ka:(bi + 1) * K, 8:16], op=mm.add)
                        tl = sml.tile([128, K * 8], F32, tag="tl")
                        nc.vector.tensor_scalar_mul(tl[:], ea[:], NEG_SLOPE)
                        nc.vector.tensor_tensor(out=ea[:], in0=ea[:], in1=tl[:],
                                                op=mm.max)
                        # ex -> overwrite the gathered a_src slots
                        nc.scalar.activation(
                            out=gmA3[:, bi * Ka:(bi + 1) * Ka, 128:136],
                            in_=ea3[:, 0:Ka, :], func=ACT.Exp)
                        nc.scalar.activation(
                            out=gmB3[:, bi * Kb:(bi + 1) * Kb, 128:136],
                            in_=ea3[:, Ka:K, :], func=ACT.Exp)
                        ps = ps1p.tile([128, 136], F32, tag="ps")
                        for k in range(K):
                            if k < Ka:
                                ck = gmA3[:, bi * Ka + k:bi * Ka + k + 1, :]
                            else:
                                kk = bi * Kb + (k - Ka)
                                ck = gmB3[:, kk:kk + 1, :]
                            S = spp.tile([128, 128], TD, tag="S")
                            nc.vector.tensor_tensor(
                                out=S[:], in0=IOTAt[:],
                                in1=dl[:, k:k + 1].to_broadcast([128, 128]),
                                op=mm.is_equal)
                            msg = ck[:, :, 0:128].rearrange(
                                "p k (h c) -> p (k h) c", c=16)
                            exb = ck[:, :, 128:136].rearrange(
                                "p k (h o) -> p (k h) o", o=1).to_broadcast(
                                [128, 8, 16])
                            nc.vector.tensor_tensor(out=msg, in0=msg, in1=exb,
                                                    op=mm.mult)
                            nc.tensor.matmul(
                                ps[:],
                                lhsT=S[:],
                                rhs=ck[:, :, 0:136].rearrange("p k w -> p (k w)"),
                                start=(k == 0), stop=(k == K - 1))
                        # epilogue: h = psum[:, :128] / den + b1 ; elu
                        rd = sml.tile([128, 8], F32, tag="rd")
                        nc.vector.tensor_scalar_add(rd[:], ps[:, 128:136], EPS)
                        nc.vector.reciprocal(rd[:], rd[:])
                        hb = hbp.tile([128, 128], F32, tag="hb")
                        nc.vector.tensor_tensor(
                            out=hb[:].rearrange("p (h c) -> p h c", c=16),
                            in0=ps[:, 0:128].rearrange("p (h c) -> p h c", c=16),
                            in1=rd[:].rearrange("p (h o) -> p h o",
                                                o=1).to_broadcast([128, 8, 16]),
                            op=mm.mult)
                        nc.vector.tensor_tensor(out=hb[:], in0=hb[:], in1=B1t[:],
                                                op=mm.add)
                        tm = hbp.tile([128, 128], F32, tag="tm")
                        nc.vector.tensor_scalar_min(tm[:], hb[:], 0.0)
                        nc.scalar.activation(out=tm[:], in_=tm[:], func=ACT.Exp)
                        nc.vector.tensor_scalar_sub(tm[:], tm[:], 1.0)
                        nc.vector.tensor_tensor(out=hb[:], in0=hb[:], in1=tm[:],
                                                op=mm.max)
                        # fused L2 node stage for this block
                        n0 = b * BLK
                        pt = psa2p.tile([128, 128], F32, tag="pa2")
                        nc.tensor.transpose(pt[:, :BLK], hb[:BLK, :],
                                            IDENT[:BLK, :BLK])
                        nc.vector.tensor_copy(out=hT[:, n0:n0 + BLK],
                                              in_=pt[:, :BLK])
                        p2m = psa2p.tile([64, 128], F32, tag="pa2")
                        nc.tensor.matmul(p2m[:, :BLK], lhsT=W2t[:],
                                         rhs=hT[:, n0:n0 + BLK],
                                         start=True, stop=True)
                        x2T = a2p.tile([64, 128], F32, tag="x2T")
                        nc.vector.tensor_copy(out=x2T[:, :BLK], in_=p2m[:, :BLK])
                        p2a = psa2p.tile([2, 128], F32, tag="pa2")
                        nc.tensor.matmul(p2a[:, :BLK], lhsT=AA2t[:],
                                         rhs=x2T[:, :BLK], start=True, stop=True)
                        a2T = a2p.tile([2, 128], F32, tag="a2T")
                        nc.vector.tensor_copy(out=a2T[:, :BLK], in_=p2a[:, :BLK])
                        p2t = psa2p.tile([128, 64], F32, tag="pa2")
                        nc.tensor.transpose(p2t[:BLK, :], x2T[:, :BLK],
                                            IDENT[:64, :64])
                        x2 = a2p.tile([128, 64], TD, tag="x2")
                        nc.vector.tensor_copy(out=x2[:BLK, :], in_=p2t[:BLK, :])
                        p2u = psa2p.tile([128, 2], F32, tag="pa2")
                        nc.tensor.transpose(p2u[:BLK, :], a2T[:, :BLK],
                                            IDENT[:2, :2])
                        a2 = a2p.tile([128, 64], TD, tag="a2")
                        nc.vector.memset(a2[:, 2:64], 0.0)
                        nc.vector.tensor_copy(out=a2[:BLK, :2], in_=p2u[:BLK, :])
                        nc.sync.dma_start(out=P2L[n0:n0 + BLK, 0:64],
                                          in_=x2[:BLK, :])
                        nc.sync.dma_start(out=P2L[n0:n0 + BLK, 64:128],
                                          in_=a2[:BLK, :])

            free_hT()

            nc.gpsimd.collective_compute(
                "AllGather", mm.bypass, replica_groups=RG,
                ins=[P2L[:, :]], outs=[P2F[:, :]])

            # ---------------- L2 edge stage ----------------
            with tc.tile_pool(name="gma2", bufs=2) as gmap2, \
                 tc.tile_pool(name="gmb2", bufs=2) as gmbp2, \
                 tc.tile_pool(name="gad2", bufs=2) as gadp2, \
                 tc.tile_pool(name="off2", bufs=2) as offp2, \
                 tc.tile_pool(name="sml2", bufs=3) as sml2, \
                 tc.tile_pool(name="sp2", bufs=3) as spp2, \
                 tc.tile_pool(name="ob", bufs=2) as obp, \
                 tc.tile_pool(name="ps2", bufs=2, space="PSUM") as ps2p:

                for b in range(NB):
                        ix = offp2.tile([128, CW], I16, tag="ix2")
                        nc.sync.dma_start(out=ix[:], in_=IDX[b, :, :])
                        gmA = gmap2.tile([128, Ka * P2W], TD, tag="gmA2")
                        gmA3 = gmA[:].rearrange("p (r w) -> p r w", w=P2W)
                        for c0 in range(0, Ka, CC):
                            c1 = min(c0 + CC, Ka)
                            nc.gpsimd.dma_gather(
                                out_ap=gmA3[:, c0:c1, :], in_ap=P2F[0:HALF, :],
                                idxs_ap=ix[:, c0 * 8:c1 * 8],
                                num_idxs=(c1 - c0) * 128,
                                num_idxs_reg=(c1 - c0) * 128, elem_size=P2W, queue_num=next_q())
                        gmB = gmbp2.tile([128, Kb * P2W], TD, tag="gmB2")
                        gmB3 = gmB[:].rearrange("p (r w) -> p r w", w=P2W)
                        for c0 in range(0, Kb, CC):
                            c1 = min(c0 + CC, Kb)
                            nc.gpsimd.dma_gather(
                                out_ap=gmB3[:, c0:c1, :],
                                in_ap=P2F[HALF:n_nodes, :],
                                idxs_ap=ix[:, (Ka + c0) * 8:(Ka + c1) * 8],
                                num_idxs=(c1 - c0) * 128,
                                num_idxs_reg=(c1 - c0) * 128, elem_size=P2W, queue_num=next_q())
                        G2E = P2W if BF16 else 64
                        G2O = 0 if BF16 else 64
                        A2C = 65 if BF16 else 1
                        gad = gadp2.tile([128, K * G2E], TD, tag="gad2")
                        gad3 = gad[:].rearrange("p (r w) -> p r w", w=G2E)
                        for c0 in range(0, K, CC):
                            c1 = min(c0 + CC, K)
                            nc.gpsimd.dma_gather(
                                out_ap=gad3[:, c0:c1, :], in_ap=P2L[:, G2O:G2O + G2E],
                                idxs_ap=ix[:, (Ka + Kb + c0) * 8:(Ka + Kb + c1) * 8],
                                num_idxs=(c1 - c0) * 128,
                                num_idxs_reg=(c1 - c0) * 128, elem_size=G2E,
                                elem_step=P2W, queue_num=next_q())
                        bi = 0
                        dl = sml2.tile([128, K], TD, tag="dl2")
                        nc.sync.dma_start(out=dl[:], in_=DLOC[b, :, :])
                        asA = gmA3[:, bi * Ka:(bi + 1) * Ka, 64:65].rearrange(
                            "p k w -> p (k w)")
                        asB = gmB3[:, bi * Kb:(bi + 1) * Kb, 64:65].rearrange(
                            "p k w -> p (k w)")
                        ea = sml2.tile([128, K], F32, tag="ea2")
                        nc.vector.tensor_tensor(
                            out=ea[:, 0:Ka], in0=asA,
                            in1=gad3[:, bi * K:bi * K + Ka, A2C:A2C + 1].rearrange(
                                "p k w -> p (k w)"), op=mm.add)
                        nc.vector.tensor_tensor(
                            out=ea[:, Ka:K], in0=asB,
                            in1=gad3[:, bi * K + Ka:(bi + 1) * K, A2C:A2C + 1].rearrange(
                                "p k w -> p (k w)"), op=mm.add)
                        tl = sml2.tile([128, K], F32, tag="tl2")
                        nc.vector.tensor_scalar_mul(tl[:], ea[:], NEG_SLOPE)
                        nc.vector.tensor_tensor(out=ea[:], in0=ea[:], in1=tl[:],
                                                op=mm.max)
                        nc.scalar.activation(out=asA, in_=ea[:, 0:Ka],
                                             func=ACT.Exp)
                        nc.scalar.activation(out=asB, in_=ea[:, Ka:K],
                                             func=ACT.Exp)
                        ps = ps2p.tile([128, 65], F32, tag="psb")
                        for k in range(K):
                            if k < Ka:
                                ck = gmA3[:, bi * Ka + k:bi * Ka + k + 1, :]
                            else:
                                kk = bi * Kb + (k - Ka)
                                ck = gmB3[:, kk:kk + 1, :]
                            ckm = ck[:, :, 0:64].rearrange("p k w -> p (k w)")
                            nc.vector.tensor_tensor(
                                out=ckm, in0=ckm,
                                in1=ck[:, :, 64:65].rearrange(
                                    "p k w -> p (k w)").to_broadcast([128, 64]),
                                op=mm.mult)
                            S = spp2.tile([128, 128], TD, tag="S2")
                            nc.vector.tensor_tensor(
                                out=S[:], in0=IOTAt[:],
                                in1=dl[:, k:k + 1].to_broadcast([128, 128]),
                                op=mm.is_equal)
                            nc.tensor.matmul(
                                ps[:],
                                lhsT=S[:],
                                rhs=ck[:, :, 0:65].rearrange("p k w -> p (k w)"),
                                start=(k == 0), stop=(k == K - 1))
                        rd = sml2.tile([128, 1], F32, tag="rd2")
                        nc.vector.tensor_scalar_add(rd[:], ps[:, 64:65], EPS)
                        nc.vector.reciprocal(rd[:], rd[:])
                        ob = obp.tile([128, 64], F32, tag="ob")
                        nc.vector.tensor_tensor(out=ob[:], in0=ps[:, 0:64],
                                                in1=rd[:].to_broadcast([128, 64]),
                                                op=mm.mult)
                        nc.vector.tensor_tensor(out=ob[:], in0=ob[:], in1=B2t[:],
                                                op=mm.add)
                        n0 = b * BLK
                        nc.sync.dma_start(out=OUT[n0:n0 + BLK, :], in_=ob[:BLK, :])

    nc.compile()
    return nc


def _run(inputs, sim=False, **rkw):
    in_maps, prm = _host_prep(**inputs)
    nc = _build_program(prm)
    n_cores = prm["n_cores"]
    if sim:
        from concourse.bass_interp import MultiCoreSim
        ms = MultiCoreSim(nc, num_cores=n_cores)
        for c in range(n_cores):
            for k, v in in_maps[c].items():
                ms.cores[c].tensor(k)[:] = v
        ms.simulate()
        outs = [np.array(ms.cores[c].tensor("out")) for c in range(n_cores)]
        return np.concatenate(outs, axis=0), None
    from concourse.bass_utils import run_bass_kernel_spmd
    res = run_bass_kernel_spmd(nc, in_maps, core_ids=list(range(n_cores)), **rkw)
    outs = [res.results[c]["out"] for c in range(n_cores)]
    return np.concatenate(outs, axis=0), res


def kernel(**inputs):
    out, _ = _run({k: np.asarray(v) for k, v in inputs.items()})
    return out

